# revision 13
# baseline (speedup 1.0000x reference)
"""Trainium2 Bass kernel for causal attention with relative-position bias.

Problem (hardcoded): B=16 heads, S=2048, Dh=64, fp32 I/O.
  dots = Q@K^T; bias pos=Q@R_w^T+R_b gathered by sign(j-i)+1; causal mask
  (-1e10 above diag); softmax(dots/sqrt(512)); out = probs@V.

Algebra: within row q the gathered bias is a constant pos0[q] for k<q and
pos1[q] at k==q (k>q masked). Softmax is invariant to per-row constants, so
only the diagonal needs exp((Q[q].(K[q]+R_w[1]-R_w[0]) + R_b[1]-R_b[0])/s).
Logits are small (|z|<=~2.2) so exp runs without max subtraction.

Layout: scores computed transposed, S^T[k,q] (k on partitions):
  S^T = (K^T tile).T @ Q^T          (lhsT=K^T[64,128], rhs=Q^T[64,ncols])
  out^T[d,q] + denominator row = [V|1].T @ exp(S^T)   (accumulated over k)
K^T stays in xbar "fold" layout (even k-tiles on SBUF partitions 0:64, odd
on 64:128); Q^T is duplicated on both partition halves.  QK matmuls for
even/odd k-tiles therefore run on disjoint PE row groups (rows 0 / 64), and
each score group pairs one even sub-fill (PSUM bank 0) with one odd sub-fill
(bank 1, tile offset 512) so consecutive LDWEIGHTS+MATMUL pairs overlap in
the array (K=64 row-tiling) without ever sharing a PSUM bank.  When the even
sub-range is narrower than 512 the gap is simply exp'd along with the data
(stale PSUM, never read downstream) - cheaper than an extra ACT call.

The diagonal 128x128 block of each k-tile is zeroed for k>=q by a DVE
multiply with a strictly-upper-triangular 0/1 mask on the exp'd slab.  The
true diagonal term pdiag[q] and its denominator contribution are added in
the epilogue in natural layout:
  out_nat[q,:] += pdiag[q] * [V|1][q,:]   (one precomputed pdv tile per head)

Scheduling: flat software pipeline over all groups of both heads: QK of
group g+2 and PV of group g-1 run while ACT exps group g (one exp call per
group).  PSUM: 3 score bufs (6 banks) + 1 outT accumulator (2 banks) = 8.
Phases run in order [1, 0]: phase 1's leading groups carry no causal masks
and only need K's first fold chunk + Q's second chunk, so the startup
critical path is one load + one cast + one fold + one unfold per operand;
unfold descriptor generation is split across the scalar+sync HWDGE queues
and head 1's loads go through the GpSimd SWDGE queue, so no DMA queue
head-of-line-blocks another.  Output phases of 1024 cols drain through
per-phase epilogues (PSUM->SBUF fp16 copy, xbar transpose back, pdv add,
reciprocal, divide, store); the final phase drains per-512 segment so the
tail is short.

Sharding: 16 heads -> 8 NeuronCores, 2 heads/core, no communication.
"""

import sys

if "/opt/trn_rl_repo" not in sys.path:
    sys.path.insert(0, "/opt/trn_rl_repo")

import numpy as np

import concourse.bacc as bacc
import concourse.mybir as mybir
import concourse.tile as tile
from concourse.bass_utils import run_bass_kernel_spmd
from concourse.masks import make_upper_triangular

B, S, DH = 16, 2048, 64
N_CORES = 8
HPC = B // N_CORES  # heads per core
P = 128
NT = S // P  # 16 q/k tiles per head
VW = 66  # V row width in SBUF: 64 values + ones col + pad (66*2B keeps 4B align)
OW = 80  # out^T rows padded to xbar multiple of 16 (64 vals + denom + 15 pad)
PH = 1024  # output phase width (outT accumulator cols)
GW = 1024  # score-group tile width
INV_SCALE = float(1.0 / np.sqrt(np.float32(512.0)))

f16 = mybir.dt.float16
f32 = mybir.dt.float32

PHASE_ORDER = (1, 0)


def build_schedule():
    """Per phase: list of groups.  Each group pairs sub-ranges of one even
    and one odd k-tile fill, [(ki, qstart, n, tile_off), ...]: even at tile
    offset 0 (<=512 cols, PSUM bank 0), odd at tile offset 512 (bank 1)."""
    phases = []
    for ph in PHASE_ORDER:
        lo, hi = ph * PH, (ph + 1) * PH
        groups = []
        for t in range(NT // 2):
            e, o = 2 * t, 2 * t + 1
            be, bo = max(P * e, lo), max(P * o, lo)
            if be >= hi:
                continue
            xs = list(range(be, hi, 512)) + [hi]
            for j in range(len(xs) - 1):
                x0, x1 = xs[j], xs[j + 1]
                g = [(e, x0, x1 - x0, 0)]
                ob = max(bo, x0)
                if ob < x1:
                    g.append((o, ob, x1 - ob, 512))
                groups.append(g)
        phases.append((ph, lo, hi, groups))
    return phases


def chunks_512(a, b):
    """Split [a, b) at multiples of 512."""
    out = []
    while a < b:
        nxt = min(b, (a // 512 + 1) * 512)
        out.append((a, nxt))
        a = nxt
    return out


def _emit(ctx, tc, q_d, k_d, v_d, rw_d, rb_d, out_d):
    nc = tc.nc
    AF = mybir.ActivationFunctionType

    const = ctx.enter_context(tc.tile_pool(name="const", bufs=1))
    ld = ctx.enter_context(tc.tile_pool(name="ld", bufs=2))
    hp = ctx.enter_context(tc.tile_pool(name="hp", bufs=2))
    slabp = ctx.enter_context(tc.tile_pool(name="slab", bufs=4))
    outp = ctx.enter_context(tc.tile_pool(name="outp", bufs=2))
    psc = ctx.enter_context(tc.tile_pool(name="psc", bufs=3, space="PSUM"))
    pout = ctx.enter_context(tc.tile_pool(name="pout", bufs=1, space="PSUM"))

    NH = NT * DH  # 1024

    # R_w rows 0/1 and R_b[0:2] broadcast to all partitions; tiny DMAs first
    # on the sync queue.
    rbc = const.tile([P, 2 * DH + 2], f32)
    nc.sync.dma_start(out=rbc[:, 0:DH], in_=rw_d[0:1, :].partition_broadcast(P))
    nc.sync.dma_start(out=rbc[:, DH : 2 * DH], in_=rw_d[1:2, :].partition_broadcast(P))
    nc.sync.dma_start(
        out=rbc[:, 2 * DH : 2 * DH + 2], in_=rb_d[None, 0:2].partition_broadcast(P)
    )

    # strictly-upper-triangular 1.0 mask (valid k<q) for diag-block zeroing;
    # first GpSimd work (pays the ext-isa IRAM load); first use is ~10 groups
    # into phase 1, long after this completes.
    m01 = const.tile([P, P], f16)
    make_upper_triangular(nc, m01[:], val=1.0, diag=False)

    st = {
        "groups": [],
        "seg_started": set(),
        "seg_stop": {},
        "kfold": {},
        "qt": {},
        "v3": {},
        "pdv": {},
        "outTs": {},
    }

    def load_chunk(eng, t32, src, h, c):
        cs = slice(c * (NH // 2), (c + 1) * (NH // 2))
        ts = slice(c * (NT // 2), (c + 1) * (NT // 2))
        eng.dma_start(
            out=t32[:, cs].rearrange("p (n d) -> p n d", d=DH),
            in_=src[h].rearrange("(n p) d -> p n d", p=P)[:, ts, :],
        )

    def head_tiles(h):
        q32 = ld.tile([P, NH], f32, tag=f"q32_{h}", bufs=1, name=f"q32_{h}")
        k32 = ld.tile([P, NH], f32, tag=f"k32_{h}", bufs=1, name=f"k32_{h}")
        v32 = ld.tile([P, NH], f32, tag=f"v32_{h}", bufs=1, name=f"v32_{h}")
        qf = hp.tile([P, NH], f16, tag="qf", name=f"qf{h}")
        kf = hp.tile([P, NH], f16, tag="kf", name=f"kf{h}")
        kfold = hp.tile([P, 8 * P], f16, tag="kfold", name=f"kfold{h}")
        qfold = hp.tile([P, 8 * P], f16, tag="qfold", name=f"qfold{h}")
        qt = hp.tile([P, S], f16, tag="qt", name=f"qt{h}")
        vaug = hp.tile([P, NT * VW], f16, tag="vaug", name=f"vaug{h}")
        return dict(q32=q32, k32=k32, v32=v32, qf=qf, kf=kf, kfold=kfold,
                    qfold=qfold, qt=qt, vaug=vaug)

    def prep_k_chunk(T, h, c):
        cs = slice(c * (NH // 2), (c + 1) * (NH // 2))
        nc.vector.tensor_copy(T["kf"][:, cs], T["k32"][:, cs])
        nc.sync.dma_start_transpose(
            out=T["kfold"][:, cs].rearrange("p (m r) -> p m r", r=P),
            in_=T["kf"][:, cs],
        )

    def prep_q_chunk(T, h, c, uengs):
        cs = slice(c * (NH // 2), (c + 1) * (NH // 2))
        ms = slice(c * 4, (c + 1) * 4)
        nc.vector.tensor_copy(T["qf"][:, cs], T["q32"][:, cs])
        nc.sync.dma_start_transpose(
            out=T["qfold"][:, cs].rearrange("p (m r) -> p m r", r=P),
            in_=T["qf"][:, cs],
        )
        qt4 = T["qt"][:].rearrange("d (m j r) -> d m j r", j=2, r=P)
        f3q = T["qfold"][:].rearrange("p (m r) -> p m r", r=P)
        uengs[0].dma_start(out=qt4[0:DH, ms, 0, :], in_=f3q[0:DH, ms])
        uengs[1].dma_start(out=qt4[0:DH, ms, 1, :], in_=f3q[DH:P, ms])
        uengs[2].dma_start(out=qt4[DH:P, ms, 0, :], in_=f3q[0:DH, ms])
        uengs[3].dma_start(out=qt4[DH:P, ms, 1, :], in_=f3q[DH:P, ms])

    def prep_v(T, h):
        v3 = T["vaug"][:].rearrange("p (n e) -> p n e", e=VW)
        nc.vector.tensor_copy(
            v3[:, :, 0:DH], T["v32"][:].rearrange("p (n d) -> p n d", d=DH)
        )
        nc.vector.memset(v3[:, :, DH : DH + 1], 1.0)
        return v3

    def prep_pre(T, h):
        """Diagonal-correction terms: pre[q] = Q[q].(K[q]+rdelta)."""
        if h == 0:
            rd16 = const.tile([P, DH], f16)
            nc.vector.tensor_sub(rd16[:], rbc[:, DH : 2 * DH], rbc[:, 0:DH])
            rbbias = const.tile([P, 1], f32)
            nc.vector.tensor_sub(
                rbbias[:], rbc[:, 2 * DH + 1 : 2 * DH + 2], rbc[:, 2 * DH : 2 * DH + 1]
            )
            nc.vector.tensor_scalar_mul(rbbias[:], rbbias[:], INV_SCALE)
            st["rd16"], st["rbbias"] = rd16, rbbias
        t2 = ld.tile([P, NH], f16, tag="t2", name=f"t2_{h}")
        t2_3 = t2[:].rearrange("p (n d) -> p n d", d=DH)
        nc.vector.tensor_add(
            t2_3,
            T["kf"][:].rearrange("p (n d) -> p n d", d=DH),
            st["rd16"][:, None, :].to_broadcast([P, NT, DH]),
        )
        nc.vector.tensor_mul(t2[:], T["qf"][:], t2[:])
        pre = hp.tile([P, NT], f32, tag="pre", name=f"pre{h}")
        nc.vector.tensor_reduce(
            out=pre[:], in_=t2_3, axis=mybir.AxisListType.X, op=mybir.AluOpType.add
        )
        return pre

    def prep_pdv(h, pre, v3):
        """pdiag = exp(pre/s + rbbias); pdv[q,:] = pdiag[q]*[V|1][q,:]."""
        pdiag = hp.tile([P, NT], f16, tag="pdiag", name=f"pdiag{h}")
        nc.scalar.activation(
            pdiag[:], pre[:], AF.Exp, bias=st["rbbias"][:, 0:1], scale=INV_SCALE
        )
        pdv = hp.tile([P, NT * (DH + 1)], f16, tag="pdv", name=f"pdv{h}")
        pdv3 = pdv[:].rearrange("p (n e) -> p n e", e=DH + 1)
        nc.vector.tensor_mul(
            pdv3,
            v3[:, :, 0 : DH + 1],
            pdiag[:, :, None].to_broadcast([P, NT, DH + 1]),
        )
        return pdv3

    # QK weights for k-tile ki come straight from the fold layout
    def kslice(kfold, ki):
        f3 = kfold[:].rearrange("p (m r) -> p m r", r=P)
        half = (ki % 2) * DH
        return f3[half : half + DH, ki // 2, :]

    def emit_qk(gi):
        G = st["groups"][gi]
        sc = psc.tile([P, GW], f32, tag="sc", name="sc")
        kfold, qt = st["kfold"][G["h"]], st["qt"][G["h"]]
        per_fill = []
        for ki, base, n, off in G["fills"]:
            half = (ki % 2) * DH
            per_fill.append(
                [
                    (ki, half, a, b, base + (a - off))
                    for a, b in chunks_512(off, off + n)
                ]
            )
        mx = max(len(c) for c in per_fill)
        for i in range(mx):
            for chunks in per_fill:
                if i < len(chunks):
                    ki, half, a, b, q0 = chunks[i]
                    nc.tensor.matmul(
                        sc[:, a:b],
                        lhsT=kslice(kfold, ki),
                        rhs=qt[half : half + DH, q0 : q0 + (b - a)],
                        start=True,
                        stop=True,
                    )
        G["sc"] = sc

    def emit_exp(gi):
        G = st["groups"][gi]
        ntot = max(f[3] + f[2] for f in G["fills"])
        slab = slabp.tile([P, GW], f16, tag="slab", name="slab")
        nc.scalar.activation(slab[:, 0:ntot], G["sc"][:, 0:ntot], AF.Exp, scale=INV_SCALE)
        G["slab"] = slab
        # zero the invalid (k>=q) half of any diagonal block
        for ki, base, n, off in G["fills"]:
            if base == P * ki:
                nc.vector.tensor_mul(
                    slab[:, off : off + P], slab[:, off : off + P], m01[:]
                )

    def emit_pv(gi):
        G = st["groups"][gi]
        slab, v3 = G["slab"], st["v3"][G["h"]]
        for ki, base, n, off in G["fills"]:
            for g0, g1 in chunks_512(base, base + n):
                key = (G["h"], G["ph"], g0 // 512)
                nc.tensor.matmul(
                    G["outT"][:, g0 - G["lo"] : g1 - G["lo"]],
                    lhsT=v3[:, ki, 0 : DH + 1],
                    rhs=slab[:, off + (g0 - base) : off + (g1 - base)],
                    start=(key not in st["seg_started"]),
                    stop=(st["seg_stop"][key] == (gi, ki, g0)),
                    skip_group_check=True,
                )
                st["seg_started"].add(key)

    def emit_epilogue(h, outT, ph_lo, lo, width):
        """Drain outT cols [lo, lo+width) -> natural layout -> HBM."""
        npm = width // P
        n0 = lo // P
        outTs = st["outTs"][h]
        nc.vector.tensor_copy(
            outTs[0 : DH + 1, lo - ph_lo : lo - ph_lo + width],
            outT[:, lo - ph_lo : lo - ph_lo + width],
        )
        onat = outp.tile([P, (PH // P) * OW], f16, tag="onat", name="onat")
        onat3 = onat[:].rearrange("p (n e) -> p n e", e=OW)[:, 0:npm]
        nc.sync.dma_start_transpose(
            out=onat3, in_=outTs[:, lo - ph_lo : lo - ph_lo + width]
        )
        onc = outp.tile([P, (PH // P) * (DH + 1)], f16, tag="onc", name="onc")
        onc3 = onc[:].rearrange("p (n e) -> p n e", e=DH + 1)[:, 0:npm]
        nc.vector.tensor_add(
            onc3, onat3[:, :, 0 : DH + 1], st["pdv"][h][:, n0 : n0 + npm, :]
        )
        recip = outp.tile([P, PH // P], f32, tag="recip", name="recip")
        nc.vector.reciprocal(recip[:, 0:npm, None], onc3[:, :, DH : DH + 1])
        ofin = outp.tile([P, (PH // P) * DH], f32, tag="ofin", name="ofin")
        ofin3 = ofin[:].rearrange("p (n d) -> p n d", d=DH)[:, 0:npm]
        nc.vector.tensor_mul(
            ofin3,
            onc3[:, :, 0:DH],
            recip[:, 0:npm, None].to_broadcast([P, npm, DH]),
        )
        nc.sync.dma_start(
            out=out_d[h].rearrange("(n p) d -> p n d", p=P)[:, n0 : n0 + npm, :],
            in_=ofin3,
        )

    # build the flat group schedule across heads+phases -------------------
    phases = build_schedule()
    for h in range(HPC):
        for ph, lo, hi, groups in phases:
            for g in groups:
                st["groups"].append(
                    {"h": h, "ph": ph, "lo": lo, "hi": hi, "fills": g}
                )
    for gi, G in enumerate(st["groups"]):
        for ki, base, n, off in G["fills"]:
            for g0, g1 in chunks_512(base, base + n):
                st["seg_stop"][(G["h"], G["ph"], g0 // 512)] = (gi, ki, g0)

    NG = len(st["groups"])
    ph_last = {}  # (h, ph) -> last group index of that phase
    for gi, G in enumerate(st["groups"]):
        ph_last[(G["h"], G["ph"])] = gi
    seg_done_at = {k: v[0] for k, v in st["seg_stop"].items()}
    last_ph_key = (HPC - 1, PHASE_ORDER[-1])

    cur_outT = {}

    def get_outT(G):
        key = (G["h"], G["ph"])
        if key not in cur_outT:
            cur_outT[key] = pout.tile([DH + 1, PH], f32, tag="outT", name="outT")
        return cur_outT[key]

    def emit_pv_and_epi(gi):
        emit_pv(gi)
        G = st["groups"][gi]
        h, ph = G["h"], G["ph"]
        if (h, ph) == last_ph_key:
            # final phase: drain per 512-col segment to shorten the tail
            for s in range(PH // 512):
                key = (h, ph, (G["lo"] + 512 * s) // 512)
                if seg_done_at[key] == gi:
                    emit_epilogue(h, G["outT"], G["lo"], G["lo"] + 512 * s, 512)
        elif gi == ph_last[(h, ph)]:
            emit_epilogue(h, G["outT"], G["lo"], G["lo"], PH)

    # ---- startup: phase 1 first => K chunk 0 and Q chunk 1 are critical ----
    T0 = head_tiles(0)
    T1 = head_tiles(1)
    load_chunk(nc.sync, T0["k32"], k_d, 0, 0)
    load_chunk(nc.sync, T0["q32"], q_d, 0, 1)
    load_chunk(nc.sync, T0["v32"], v_d, 0, 0)
    prep_k_chunk(T0, 0, 0)
    # split unfold descriptor generation across the scalar + sync HWDGEs
    prep_q_chunk(T0, 0, 1, [nc.scalar, nc.scalar, nc.sync, nc.sync])
    load_chunk(nc.sync, T0["v32"], v_d, 0, 1)
    load_chunk(nc.sync, T0["k32"], k_d, 0, 1)
    load_chunk(nc.sync, T0["q32"], q_d, 0, 0)
    prep_k_chunk(T0, 0, 1)
    prep_q_chunk(T0, 0, 0, [nc.sync, nc.sync, nc.sync, nc.sync])
    v30 = prep_v(T0, 0)
    st["kfold"][0], st["qt"][0], st["v3"][0] = T0["kfold"], T0["qt"], v30
    pre0 = prep_pre(T0, 0)
    outTs0 = outp.tile([OW, PH], f16, tag="outTs", name="outTs0")
    nc.vector.memset(outTs0[DH : OW, :], 0.0)
    st["outTs"][0] = outTs0
    # head 1 loads via the GpSimd SWDGE queue (after m01's ext-isa work)
    for c in range(2):
        load_chunk(nc.gpsimd, T1["k32"], k_d, 1, c)
        load_chunk(nc.gpsimd, T1["q32"], q_d, 1, c)
        load_chunk(nc.gpsimd, T1["v32"], v_d, 1, c)

    # ---- flat pipeline: ACT exps group g while PE runs QK(g+2) + PV(g-1) ----
    st["groups"][0]["outT"] = get_outT(st["groups"][0])
    emit_qk(0)
    emit_qk(1)

    for gi in range(NG):
        G = st["groups"][gi]
        G["outT"] = get_outT(G)
        emit_exp(gi)

        # deferred prep work, interleaved into the pipeline
        if gi == 2:
            st["pdv"][0] = prep_pdv(0, pre0, v30)
        if gi == 7:
            prep_k_chunk(T1, 1, 0)
            prep_q_chunk(T1, 1, 1, [nc.sync] * 4)
            prep_k_chunk(T1, 1, 1)
            prep_q_chunk(T1, 1, 0, [nc.sync] * 4)
            v31 = prep_v(T1, 1)
            st["kfold"][1], st["qt"][1], st["v3"][1] = T1["kfold"], T1["qt"], v31
            st["pre1"] = prep_pre(T1, 1)
            outTs1 = outp.tile([OW, PH], f16, tag="outTs", name="outTs1")
            nc.vector.memset(outTs1[DH : OW, :], 0.0)
            st["outTs"][1] = outTs1
        if gi == 10:
            st["pdv"][1] = prep_pdv(1, st["pre1"], st["v3"][1])

        if gi + 2 < NG:
            emit_qk(gi + 2)
        if gi > 0:
            emit_pv_and_epi(gi - 1)

    emit_pv_and_epi(NG - 1)


def build_nc(debug=False):
    from contextlib import ExitStack

    nc = bacc.Bacc("TRN2", target_bir_lowering=False, debug=debug, num_devices=N_CORES)
    q_d = nc.dram_tensor("query", [HPC, S, DH], f32, kind="ExternalInput").ap()
    k_d = nc.dram_tensor("key", [HPC, S, DH], f32, kind="ExternalInput").ap()
    v_d = nc.dram_tensor("value", [HPC, S, DH], f32, kind="ExternalInput").ap()
    rw_d = nc.dram_tensor("R_w", [3, DH], f32, kind="ExternalInput").ap()
    rb_d = nc.dram_tensor("R_b", [3], f32, kind="ExternalInput").ap()
    out_d = nc.dram_tensor("out", [HPC, S, DH], f32, kind="ExternalOutput").ap()
    with tile.TileContext(nc) as tc, ExitStack() as ctx:
        _emit(ctx, tc, q_d, k_d, v_d, rw_d, rb_d, out_d)
    nc.finalize()
    return nc


_NC_CACHE = {}


def _get_nc():
    if "nc" not in _NC_CACHE:
        _NC_CACHE["nc"] = build_nc()
    return _NC_CACHE["nc"]


def kernel(query, key, value, R_w, R_b, trace=False):
    query = np.ascontiguousarray(np.asarray(query, dtype=np.float32))
    key = np.ascontiguousarray(np.asarray(key, dtype=np.float32))
    value = np.ascontiguousarray(np.asarray(value, dtype=np.float32))
    R_w = np.ascontiguousarray(np.asarray(R_w, dtype=np.float32))
    R_b = np.ascontiguousarray(np.asarray(R_b, dtype=np.float32))

    nc = _get_nc()
    in_maps = [
        {
            "query": query[c * HPC : (c + 1) * HPC],
            "key": key[c * HPC : (c + 1) * HPC],
            "value": value[c * HPC : (c + 1) * HPC],
            "R_w": R_w,
            "R_b": R_b,
        }
        for c in range(N_CORES)
    ]
    res = run_bass_kernel_spmd(nc, in_maps, core_ids=list(range(N_CORES)), trace=trace)
    out = np.concatenate([res.results[c]["out"] for c in range(N_CORES)], axis=0)
    if trace:
        kernel.last_results = res
    return out.astype(np.float32, copy=False)
